# revision 45
# baseline (speedup 1.0000x reference)
"""Causal self-attention (single head) on 8 TRN2 NeuronCores — v4.

Reference: q/k/v = x @ W* + b*  (x: [4,4096,1024], W: [1024,64])
           att = softmax(mask(q k^T / sqrt(1024)));  out = att @ v

Sharding: batch b -> core pair {b, b+4}; core b takes query chunks
{0,1,6,7}, core b+4 takes {2,3,4,5} (both 72 causal key-blocks).  k/v
are computed per-core (replicated); no collectives.

v4: the kernel is a software-pipelined stream of 36 half-units per
core.  A half-unit = [2 row-packed score matmuls -> exp (ScalarE) ->
triangle mask (diag only) -> 2 PV matmuls].  The PV of half-unit m is
emitted AFTER the score matmuls of half-unit m+1, so the PE FIFO is
[s_m, pv_{m-1}, filler] per exp call and the exp stream never waits on
a projection burst (v2/v3 lost ~1.7us per kv round to exactly that).
All other PE work (kv projections in 2-matmul slices, the second q
pair, V transposes, epilogues) is spread between half-units as fillers
sized under the per-half ScalarE budget (~1.1us).  ~50 identity
matmuls run during the initial DMA window to hold the PE HAM
clock-gate at 2.4GHz.  All DMA issues live on the GpSimd queue;
ScalarE runs nothing but exps.  Biases are zero for this problem
(checked host-side, numpy fallback otherwise) so PSUM evacuations are
plain copies.

All matmuls in bf16; PSUM fp32.  Scores are computed transposed
(S^T = K Q^T) so softmax needs no max pass (logits are tiny), exp runs
straight out of PSUM, and PV with a ones-augmented V accumulates both
the output numerator and the softmax denominator in one PSUM tile per
query group.  A final PE transpose + reciprocal normalize produces the
output.
"""

import sys
import types

sys.path.insert(0, "/opt/trn_rl_repo")

import numpy as np

B, T, D, H = 4, 4096, 1024, 64
NCORE = 8
TCH = 512
NCH = T // TCH                 # 8 chunks
JB = 128                       # key block
SCALE = 1.0 / 32.0             # 1/sqrt(D)
EVEN_CHUNKS = (0, 1, 6, 7)
ODD_CHUNKS = (2, 3, 4, 5)

# slot s of xT holds chunk LOAD[s]; DMA issues in ascending slot order.
EVEN_LOAD = (0, 1, 6, 7, 2, 3, 4, 5)
ODD_LOAD = (0, 2, 3, 1, 4, 5, 6, 7)    # slots 6,7 unused on odd cores
EVEN_QSLOTS = (0, 1, 2, 3)     # slot of q chunk groups[i]
ODD_QSLOTS = (1, 2, 4, 5)
EVEN_NLOAD = 8
ODD_NLOAD = 6

# unit processing order (group index, kv chunk index), interleaving the
# two PSUM-accumulator chains (B: g0 then g2 / A: g1 then g3) in
# kv-arrival order.
UO_EVEN = [(0, 0), (1, 0), (1, 1), (2, 0), (3, 0), (2, 1), (3, 1),
           (2, 2), (3, 2), (2, 3), (3, 3), (2, 4), (3, 4), (2, 5),
           (3, 5), (2, 6), (3, 6), (3, 7)]
UO_ODD = [(0, 0), (1, 0), (0, 1), (1, 1), (0, 2), (1, 2), (2, 0),
          (1, 3), (2, 1), (3, 0), (2, 2), (3, 1), (2, 3), (3, 2),
          (2, 4), (3, 3), (3, 4), (3, 5)]

# emitted before the half-unit stream on every core
PRE_SCHED = [('kv0',), ('kve', 0), ('w2',), ('qp', 0, 1, 0),
             ('qp', 0, 1, 1), ('qp', 0, 1, 2), ('qp', 0, 1, 3),
             ('qe', 0, 1), ('vfin', 0), ('kvm', 1, 0), ('kvm', 1, 1)]

# fillers emitted after half-unit index i (hand-packed against each
# item's data-arrival time, first-consumer deadline, aux-pool rotation,
# and the ~1.1us/half PE budget: one 2-matmul item per half mid-stream)
FILL_EVEN = {
    0: [('kvm', 1, 2), ('kvm', 1, 3)],
    1: [('kve', 1), ('qp', 2, 3, 0)],
    2: [('vfin', 1), ('qp', 2, 3, 1)],
    3: [('qp', 2, 3, 2), ('qp', 2, 3, 3)],
    4: [('qe', 2, 3)],
    5: [('epi', 0), ('kvm', 2, 0)],
    6: [('epi', 1), ('kvm', 2, 1)],
    7: [('kvm', 2, 2)],
    8: [('kvm', 2, 3), ('kve', 2)],
    9: [('vfin', 2)],
    10: [('kvm', 3, 0)], 11: [('kvm', 3, 1)], 12: [('kvm', 3, 2)],
    13: [('kvm', 3, 3), ('kve', 3)], 14: [('vfin', 3)],
    15: [('kvm', 4, 0)], 16: [('kvm', 4, 1)], 17: [('kvm', 4, 2)],
    18: [('kvm', 4, 3), ('kve', 4)], 19: [('vfin', 4)],
    20: [('kvm', 5, 0)], 21: [('kvm', 5, 1)], 22: [('kvm', 5, 2)],
    23: [('kvm', 5, 3), ('kve', 5)], 24: [('vfin', 5)],
    25: [('kvm', 6, 0)], 26: [('kvm', 6, 1)], 27: [('kvm', 6, 2)],
    28: [('kvm', 6, 3), ('kve', 6)], 29: [('vfin', 6)],
    30: [('kvm', 7, 0)], 31: [('kvm', 7, 1)], 32: [('kvm', 7, 2)],
    33: [('kvm', 7, 3), ('kve', 7)], 34: [('vfin', 7), ('epi', 2)],
}
FILL_ODD = {
    0: [('kvm', 1, 2), ('kvm', 1, 3)],
    1: [('kve', 1)],
    2: [('vfin', 1)],
    3: [('kvm', 2, 0)],
    4: [('kvm', 2, 1), ('qp', 2, 3, 0)],
    5: [('kvm', 2, 2), ('qp', 2, 3, 1)],
    6: [('kvm', 2, 3), ('kve', 2)],
    7: [('vfin', 2), ('qp', 2, 3, 2)],
    8: [('qp', 2, 3, 3), ('qe', 2, 3)],
    9: [('kvm', 3, 0)],
    10: [('epi', 0), ('kvm', 3, 1)],
    11: [('kvm', 3, 2)],
    12: [('kvm', 3, 3), ('kve', 3)],
    13: [('vfin', 3)],
    16: [('epi', 1)],
    17: [('kvm', 4, 0)], 18: [('kvm', 4, 1)], 19: [('kvm', 4, 2)],
    20: [('kvm', 4, 3), ('kve', 4)], 21: [('vfin', 4)],
    22: [('kvm', 5, 0)], 23: [('kvm', 5, 1)], 24: [('kvm', 5, 2)],
    25: [('kvm', 5, 3), ('kve', 5)], 26: [('vfin', 5)],
    30: [('epi', 2)],
}


def _install_profile_hook():
    """Best-effort NTFF profiling hook (the image's antenv lacks axon_hooks)."""
    try:
        import antenv
        if "antenv.axon_hooks" in sys.modules:
            return
        hooks_mod = types.ModuleType("antenv.axon_hooks")
        _h = [None]
        hooks_mod.set_axon_ntff_profile_hook = lambda h: _h.__setitem__(0, h)
        hooks_mod.get_axon_ntff_profile_hook = lambda: _h[0]
        sys.modules["antenv.axon_hooks"] = hooks_mod
        antenv.axon_hooks = hooks_mod
        from trn_agent_boot.trn_boot import _ntff_profile_via_ctypes
        hooks_mod.set_axon_ntff_profile_hook(
            _ntff_profile_via_ctypes("/opt/axon/libaxon_pjrt.so")
        )
        import concourse.bass_utils as bass_utils
        bass_utils.upload_artifacts = lambda tmpdir: f"local:{tmpdir}"
    except Exception:
        pass


def build_graph():
    import concourse.bacc as bacc
    import concourse.mybir as mybir
    import concourse.tile as tile
    from concourse import masks

    F32 = mybir.dt.float32
    BF16 = mybir.dt.bfloat16

    nc = bacc.Bacc("TRN2", target_bir_lowering=False, debug=False,
                   num_devices=NCORE)

    xT = nc.dram_tensor("xT", [NCH, 128, 8, TCH], BF16,
                        kind="ExternalInput").ap()
    wkv = nc.dram_tensor("wkv", [128, 8, 2 * H], BF16,
                         kind="ExternalInput").ap()
    wq = nc.dram_tensor("wq", [128, 8, H], BF16, kind="ExternalInput").ap()
    # numerator (64) + denominator (col 64) + pad, normalized on host
    out = nc.dram_tensor("out", [T // 2, H + 2], BF16,
                         kind="ExternalOutput").ap()
    out_r = out.rearrange("(l p) h -> p l h", p=128)

    with tile.TileContext(nc) as tc:
        import contextlib
        with contextlib.ExitStack() as ctx:
            _body(ctx, tc, nc, mybir, masks, xT, wkv, wq, out_r)

    nc.compile()
    return nc


def _body(ctx, tc, nc, mybir, masks, xT, wkv, wq, out_r):
    F32 = mybir.dt.float32
    BF16 = mybir.dt.bfloat16
    Exp = mybir.ActivationFunctionType.Exp

    const = ctx.enter_context(tc.tile_pool(name="const", bufs=1))
    big = ctx.enter_context(tc.tile_pool(name="big", bufs=1))
    vs_pool = ctx.enter_context(tc.tile_pool(name="vs", bufs=2))
    pt_pool = ctx.enter_context(tc.tile_pool(name="pt", bufs=6))
    wk_pool = ctx.enter_context(tc.tile_pool(name="wk", bufs=3))
    ps_pool = ctx.enter_context(tc.tile_pool(name="ps", bufs=2, space="PSUM"))
    po_pool = ctx.enter_context(tc.tile_pool(name="po", bufs=2, space="PSUM"))
    aux_ps = ctx.enter_context(tc.tile_pool(name="auxps", bufs=2, space="PSUM"))

    _psn = [0]
    def ps_tile(shape, dtype=None):
        _psn[0] += 1
        return ps_pool.tile(shape, dtype or F32, tag="ps", name=f"ps{_psn[0]}")

    def po_tile(shape, dtype=None):
        _psn[0] += 1
        return po_pool.tile(shape, dtype or F32, tag="po", name=f"po{_psn[0]}")

    def aux_tile(shape, dtype=None):
        _psn[0] += 1
        return aux_ps.tile(shape, dtype or F32, tag="aux", name=f"aux{_psn[0]}")

    _wkn = [0]
    def wk_tile(shape, dtype=None, tag="wk"):
        _wkn[0] += 1
        return wk_pool.tile(shape, dtype or F32, tag=tag,
                            name=f"{tag}{_wkn[0]}")

    # ---- identity first (gates the PE warmup), then weight DMAs, then
    # the first x chunk, then remaining constants and x chunks.  All
    # issue from the GpSimd queue.
    ident = const.tile([128, 128], BF16)
    masks.make_identity(nc, ident[:])

    kT_a = big.tile([128, NCH, TCH], BF16)
    vA_a = big.tile([128, NCH, 4, H + 1], BF16)
    qT_a = big.tile([128, 4, TCH], BF16)
    xcs = [big.tile([128, 8, TCH], BF16, name=f"xc{s}") for s in range(NCH)]

    # Each dma_start binds to one ~23GB/s DMA engine, so aggregate
    # bandwidth ramps with the number of transfers in flight: fan the
    # issues across four engine queues in parallel and split the
    # early-needed data fine.  Priority order: weights, chunk 0, 1, ...
    # transfers with 4KB-per-partition lines sustain ~410GB/s aggregate
    # (finer splits drop to ~250); priority order = weights, slot 0, 1, ...
    w_kv = const.tile([128, 8, 2 * H], BF16)
    w_q = const.tile([128, 8, H], BF16)
    pieces = [(w_kv[:], wkv[:]), (w_q[:], wq[:])]
    for s in range(2):                       # first two slots: quarters
        for q4 in range(4):
            pieces.append((xcs[s][:, 2 * q4:2 * q4 + 2, :],
                           xT[s][:, 2 * q4:2 * q4 + 2, :]))
    for s in range(2, 4):
        pieces.append((xcs[s][:, 0:4, :], xT[s][:, 0:4, :]))
        pieces.append((xcs[s][:, 4:8, :], xT[s][:, 4:8, :]))
    queues = [nc.gpsimd, nc.scalar, nc.sync]
    for idx, (dst, src) in enumerate(pieces):
        queues[idx % 3].dma_start(dst, src)

    ones_col = const.tile([128, 4], BF16)
    nc.gpsimd.memset(ones_col[:], 1.0)
    # triangle mask for the diagonal 128x128 sub-blocks
    tri = const.tile([128, JB], BF16)
    nc.gpsimd.memset(tri[:], 1.0)
    nc.gpsimd.affine_select(
        out=tri[:], in_=tri[:], compare_op=mybir.AluOpType.is_ge,
        fill=0.0, base=0, channel_multiplier=-1, pattern=[[1, JB]])

    # PE HAM warmup across the DMA window
    warm = ps_tile([128, 2 * TCH])
    for _ in range(48):
        nc.tensor.matmul(warm[:, 0:128], ident[:], ident[:],
                         start=True, stop=True)

    def branch(load_order, q_chunks, q_slots, n_load, unit_order, fillers):
        slot_of = {c: s for s, c in enumerate(load_order)}
        groups = sorted(q_chunks)

        pkv_of, pq_of, vt_of, po_of = {}, {}, {}, {}
        pending = [None]           # (group, pv_closure)

        def kvm(c, j):
            if j == 0:
                pkv_of[c] = aux_tile([128, TCH])
            p, s = pkv_of[c], slot_of[c]
            for dc in (2 * j, 2 * j + 1):
                nc.tensor.matmul(p[:], w_kv[:, dc, :], xcs[s][:, dc, :],
                                 start=(dc == 0), stop=(dc == 7))

        def kve(c):
            p, s = pkv_of[c], slot_of[c]
            nc.vector.tensor_copy(kT_a[0:64, s, :], p[0:64, :])
            nc.vector.tensor_copy(kT_a[64:128, s, :], kT_a[0:64, s, :])
            vt_of[c] = vs_pool.tile([64, TCH], BF16, name=f"vt{s}", tag="vt")
            nc.vector.tensor_copy(vt_of[c][:], p[64:128, :])

        def vfin(c):
            s = slot_of[c]
            ptrv = aux_tile([128, 4, H], BF16)
            for jj in range(4):
                nc.tensor.transpose(ptrv[:, jj, :],
                                    vt_of[c][:, jj * 128:(jj + 1) * 128],
                                    ident[0:64, 0:64])
            nc.vector.tensor_copy(vA_a[:, s, :, 0:H], ptrv[:])
            nc.vector.tensor_copy(vA_a[:, s, :, H], ones_col[:, 0:4])

        def qp(a, b, p):
            if p == 0:
                pq_of[(a, b)] = aux_tile([128, TCH])
            q = pq_of[(a, b)]
            for dc in range(2 * p, 2 * p + 2):
                nc.tensor.matmul(q[0:64, :], w_q[:, dc, :],
                                 xcs[q_slots[a]][:, dc, :],
                                 start=(dc == 0), stop=(dc == 7),
                                 tile_position=(0, 0), skip_group_check=True)
                nc.tensor.matmul(q[64:128, :], w_q[:, dc, :],
                                 xcs[q_slots[b]][:, dc, :],
                                 start=(dc == 0), stop=(dc == 7),
                                 tile_position=(0, 64), skip_group_check=True)

        def qe(a, b):
            q = pq_of[(a, b)]
            for g, lo in ((a, 0), (b, 64)):
                nc.vector.tensor_copy(qT_a[0:64, g, :], q[lo:lo + 64, :])
                nc.vector.tensor_copy(qT_a[64:128, g, :], qT_a[0:64, g, :])

        def flush_pv():
            if pending[0] is None:
                return
            _, pv = pending[0]
            pending[0] = None
            pv()

        def do_half(g, cu, hh):
            sj = slot_of[cu]
            diag = (cu == groups[g])
            if cu == 0 and hh == 0:
                po_of[g] = po_tile([H + 1, TCH])
            po = po_of[g]
            pp = ps_tile([128, 2 * TCH])
            for k_ in range(2):
                t_ = 2 * hh + k_
                lo = k_ * 64
                nc.tensor.matmul(
                    pp[:, k_ * TCH:(k_ + 1) * TCH],
                    kT_a[lo:lo + 64, sj, t_ * 128:(t_ + 1) * 128],
                    qT_a[lo:lo + 64, g, :],
                    start=True, stop=True, tile_position=(lo, 0))
            pt = pt_pool.tile([128, 2 * TCH], BF16)
            if diag and hh == 1:
                # only the causally-live column ranges of blocks t2,t3
                nc.scalar.activation(pt[:, 256:512], pp[:, 256:512],
                                     Exp, scale=SCALE)
                nc.scalar.activation(pt[:, 896:1024], pp[:, 896:1024],
                                     Exp, scale=SCALE)
            else:
                nc.scalar.activation(pt[:], pp[:], Exp, scale=SCALE)
            if diag:
                for k_ in range(2):
                    t_ = 2 * hh + k_
                    c0 = t_ * JB
                    nc.vector.tensor_mul(
                        pt[:, k_ * TCH + c0:k_ * TCH + c0 + JB],
                        pt[:, k_ * TCH + c0:k_ * TCH + c0 + JB], tri[:])

            def pv():
                for k_ in range(2):
                    t_ = 2 * hh + k_
                    c0 = t_ * JB if diag else 0
                    nc.tensor.matmul(po[:, c0:TCH], vA_a[:, sj, t_, :],
                                     pt[:, k_ * TCH + c0:(k_ + 1) * TCH],
                                     start=(cu == 0 and t_ == 0),
                                     stop=(cu == groups[g] and t_ == 3),
                                     skip_group_check=True)
            return (g, pv)

        def epilogue(g):
            # ship numerator + denominator; the host does the divide
            po = po_of[g]
            ot = wk_tile([H + 1, TCH], BF16, tag="ot")
            nc.vector.tensor_copy(ot[:], po[:])
            ptr2 = aux_tile([128, 4, H + 2], BF16)
            for jj in range(4):
                nc.tensor.transpose(ptr2[:, jj, 0:H + 1],
                                    ot[:, jj * 128:(jj + 1) * 128],
                                    ident[0:H + 1, 0:H + 1])
            og = wk_tile([128, 4, H + 2], BF16, tag="og")
            nc.vector.tensor_copy(og[:], ptr2[:])
            nc.sync.dma_start(out_r[:, g * 4:(g + 1) * 4, :], og[:])

        def do_op(op):
            if op[0] == 'kv0':
                kvm(0, 0); kvm(0, 1); kvm(0, 2); kvm(0, 3)
            elif op[0] == 'kvm':
                kvm(op[1], op[2])
            elif op[0] == 'kve':
                kve(op[1])
            elif op[0] == 'vfin':
                vfin(op[1])
            elif op[0] == 'qp':
                qp(op[1], op[2], op[3])
            elif op[0] == 'qe':
                qe(op[1], op[2])
            elif op[0] == 'w2':
                for s in range(4, n_load):
                    # real data dependency (reads kT chunk 0) so the
                    # scheduler cannot hoist these issues into the
                    # critical early-DMA window and dilute slots 0-3
                    nc.gpsimd.tensor_copy(xcs[s][0:1, 0:1, 0:4],
                                          kT_a[0:1, slot_of[0], 0:4])
                    nc.gpsimd.dma_start(xcs[s][:, 0:4, :], xT[s][:, 0:4, :])
                    nc.gpsimd.dma_start(xcs[s][:, 4:8, :], xT[s][:, 4:8, :])
            elif op[0] == 'epi':
                if pending[0] is not None and pending[0][0] == op[1]:
                    flush_pv()
                epilogue(op[1])

        for op in PRE_SCHED:
            do_op(op)
        hidx = 0
        for (g, cu) in unit_order:
            for hh in range(2):
                new_pv = do_half(g, cu, hh)
                for op in fillers.get(hidx, []):
                    do_op(op)
                flush_pv()
                pending[0] = new_pv
                hidx += 1
        flush_pv()
        epilogue(3)

    pid = nc.partition_id()
    with tc.If(pid < 4) as cmp:
        branch(EVEN_LOAD, EVEN_CHUNKS, EVEN_QSLOTS, EVEN_NLOAD,
               UO_EVEN, FILL_EVEN)
    with cmp.Else():
        branch(ODD_LOAD, ODD_CHUNKS, ODD_QSLOTS, ODD_NLOAD,
               UO_ODD, FILL_ODD)


_GRAPH = None


def _get_graph():
    global _GRAPH
    if _GRAPH is None:
        _install_profile_hook()
        _GRAPH = build_graph()
    return _GRAPH


def _in_maps(x, Wq, Wk, Wv):
    import ml_dtypes
    bf16 = ml_dtypes.bfloat16
    x = np.asarray(x, np.float32)
    wkv = np.concatenate([np.asarray(Wk, np.float32),
                          np.asarray(Wv, np.float32)], axis=1).astype(bf16)
    wkv = wkv.reshape(8, 128, 2 * H).transpose(1, 0, 2).copy()
    wq = np.asarray(Wq, np.float32).astype(bf16)
    wq = wq.reshape(8, 128, H).transpose(1, 0, 2).copy()
    maps = []
    for c in range(NCORE):
        b = c % B
        order = EVEN_LOAD if c < 4 else ODD_LOAD
        n_load = EVEN_NLOAD if c < 4 else ODD_NLOAD
        xb = x[b]                                    # [T, D]
        xTc = np.zeros((NCH, 128, 8, TCH), bf16)
        for s, gc in enumerate(order):
            if s >= n_load:
                continue
            ch = xb[gc * TCH:(gc + 1) * TCH].T       # [D, TCH]
            xTc[s] = ch.reshape(8, 128, TCH).transpose(1, 0, 2)
        maps.append({"xT": xTc, "wkv": wkv, "wq": wq})
    return maps


def _unshard(results):
    out = np.empty((B, T, H), np.float32)
    for c in range(NCORE):
        b = c % B
        chunks = EVEN_CHUNKS if c < 4 else ODD_CHUNKS
        o = np.asarray(results[c]["out"], np.float32)  # [2048, 66] num+den
        ob = o[:, 0:H] / o[:, H:H + 1]
        for g, gc in enumerate(sorted(chunks)):
            out[b, gc * TCH:(gc + 1) * TCH] = ob[g * TCH:(g + 1) * TCH]
    return out


def run_spmd(inputs, trace=False):
    """Run on 8 cores; returns (output, BassKernelResults)."""
    from concourse.bass_utils import run_bass_kernel_spmd
    nc = _get_graph()
    maps = _in_maps(inputs["x"], inputs["Wq"], inputs["Wk"], inputs["Wv"])
    res = run_bass_kernel_spmd(nc, maps, core_ids=list(range(NCORE)),
                               trace=trace)
    return _unshard(res.results), res


def _numpy_fallback(x, Wq, bq, Wk, bk, Wv, bv):
    x = np.asarray(x, np.float32)
    q = x @ Wq + bq
    k = x @ Wk + bk
    v = x @ Wv + bv
    att = np.einsum("bth,bsh->bts", q, k) / np.sqrt(np.float32(D))
    causal = np.tril(np.ones((T, T), dtype=bool))
    att = np.where(causal, att, -np.inf)
    att = att - att.max(axis=-1, keepdims=True)
    e = np.exp(att)
    att = e / e.sum(axis=-1, keepdims=True)
    return np.einsum("bts,bsh->bth", att, v).astype(np.float32)


def kernel(x, Wq, bq, Wk, bk, Wv, bv):
    if np.any(np.asarray(bq)) or np.any(np.asarray(bk)) \
            or np.any(np.asarray(bv)):
        return _numpy_fallback(x, Wq, bq, Wk, bk, Wv, bv)
    out, _ = run_spmd(dict(x=x, Wq=Wq, Wk=Wk, Wv=Wv))
    return out


# revision 47
# speedup vs baseline: 1.0389x; 1.0389x over previous
"""Causal self-attention (single head) on 8 TRN2 NeuronCores — v4.

Reference: q/k/v = x @ W* + b*  (x: [4,4096,1024], W: [1024,64])
           att = softmax(mask(q k^T / sqrt(1024)));  out = att @ v

Sharding: batch b -> core pair {b, b+4}; core b takes query chunks
{0,1,6,7}, core b+4 takes {2,3,4,5} (both 72 causal key-blocks).  k/v
are computed per-core (replicated); no collectives.

v4: the kernel is a software-pipelined stream of 36 half-units per
core.  A half-unit = [2 row-packed score matmuls -> exp (ScalarE) ->
triangle mask (diag only) -> 2 PV matmuls].  The PV of half-unit m is
emitted AFTER the score matmuls of half-unit m+1, so the PE FIFO is
[s_m, pv_{m-1}, filler] per exp call and the exp stream never waits on
a projection burst (v2/v3 lost ~1.7us per kv round to exactly that).
All other PE work (kv projections in 2-matmul slices, the second q
pair, V transposes, epilogues) is spread between half-units as fillers
sized under the per-half ScalarE budget (~1.1us).  ~50 identity
matmuls run during the initial DMA window to hold the PE HAM
clock-gate at 2.4GHz.  All DMA issues live on the GpSimd queue;
ScalarE runs nothing but exps.  Biases are zero for this problem
(checked host-side, numpy fallback otherwise) so PSUM evacuations are
plain copies.

All matmuls in bf16; PSUM fp32.  Scores are computed transposed
(S^T = K Q^T) so softmax needs no max pass (logits are tiny), exp runs
straight out of PSUM, and PV with a ones-augmented V accumulates both
the output numerator and the softmax denominator in one PSUM tile per
query group.  A final PE transpose + reciprocal normalize produces the
output.
"""

import sys
import types

sys.path.insert(0, "/opt/trn_rl_repo")

import numpy as np

B, T, D, H = 4, 4096, 1024, 64
NCORE = 8
TCH = 512
NCH = T // TCH                 # 8 chunks
JB = 128                       # key block
SCALE = 1.0 / 32.0             # 1/sqrt(D)
EVEN_CHUNKS = (0, 1, 6, 7)
ODD_CHUNKS = (2, 3, 4, 5)

# slot s of xT holds chunk LOAD[s]; DMA issues in ascending slot order.
EVEN_LOAD = (0, 1, 6, 7, 2, 3, 4, 5)
ODD_LOAD = (0, 2, 3, 1, 4, 5, 6, 7)    # slots 6,7 unused on odd cores
EVEN_QSLOTS = (0, 1, 2, 3)     # slot of q chunk groups[i]
ODD_QSLOTS = (1, 2, 4, 5)
EVEN_NLOAD = 8
ODD_NLOAD = 6

# unit processing order (group index, kv chunk index), interleaving the
# two PSUM-accumulator chains (B: g0 then g2 / A: g1 then g3) in
# kv-arrival order.
UO_EVEN = [(0, 0), (1, 0), (1, 1), (2, 0), (3, 0), (2, 1), (3, 1),
           (2, 2), (3, 2), (2, 3), (3, 3), (2, 4), (3, 4), (2, 5),
           (3, 5), (2, 6), (3, 6), (3, 7)]
UO_ODD = [(0, 0), (1, 0), (0, 1), (1, 1), (0, 2), (1, 2), (2, 0),
          (1, 3), (2, 1), (3, 0), (2, 2), (3, 1), (2, 3), (3, 2),
          (2, 4), (3, 3), (3, 4), (3, 5)]

# emitted before the half-unit stream on every core
PRE_SCHED = [('kv0',), ('kve', 0), ('w2',), ('qp', 0, 1, 0),
             ('qp', 0, 1, 1), ('qp', 0, 1, 2), ('qp', 0, 1, 3),
             ('qe', 0, 1), ('vfin', 0), ('kvm', 1, 0), ('kvm', 1, 1)]

# fillers emitted after half-unit index i (hand-packed against each
# item's data-arrival time, first-consumer deadline, aux-pool rotation,
# and the ~1.1us/half PE budget: one 2-matmul item per half mid-stream)
FILL_EVEN = {
    0: [('kvm', 1, 2), ('kvm', 1, 3)],
    1: [('kve', 1), ('qp', 2, 3, 0)],
    2: [('vfin', 1), ('qp', 2, 3, 1)],
    3: [('qp', 2, 3, 2), ('qp', 2, 3, 3)],
    4: [('qe', 2, 3)],
    5: [('epi', 0), ('kvm', 2, 0)],
    6: [('epi', 1), ('kvm', 2, 1)],
    7: [('kvm', 2, 2)],
    8: [('kvm', 2, 3), ('kve', 2)],
    9: [('vfin', 2)],
    10: [('kvm', 3, 0)], 11: [('kvm', 3, 1)], 12: [('kvm', 3, 2)],
    13: [('kvm', 3, 3), ('kve', 3)], 14: [('vfin', 3)],
    15: [('kvm', 4, 0)], 16: [('kvm', 4, 1)], 17: [('kvm', 4, 2)],
    18: [('kvm', 4, 3), ('kve', 4)], 19: [('vfin', 4)],
    20: [('kvm', 5, 0)], 21: [('kvm', 5, 1)], 22: [('kvm', 5, 2)],
    23: [('kvm', 5, 3), ('kve', 5)], 24: [('vfin', 5)],
    25: [('kvm', 6, 0)], 26: [('kvm', 6, 1)], 27: [('kvm', 6, 2)],
    28: [('kvm', 6, 3), ('kve', 6)], 29: [('vfin', 6)],
    30: [('kvm', 7, 0)], 31: [('kvm', 7, 1)], 32: [('kvm', 7, 2)],
    33: [('kvm', 7, 3), ('kve', 7)], 34: [('vfin', 7), ('epi', 2)],
}
FILL_ODD = {
    0: [('kvm', 1, 2), ('kvm', 1, 3)],
    1: [('kve', 1)],
    2: [('vfin', 1)],
    3: [('kvm', 2, 0)],
    4: [('kvm', 2, 1), ('qp', 2, 3, 0)],
    5: [('kvm', 2, 2), ('qp', 2, 3, 1)],
    6: [('kvm', 2, 3), ('kve', 2)],
    7: [('vfin', 2), ('qp', 2, 3, 2)],
    8: [('qp', 2, 3, 3), ('qe', 2, 3)],
    9: [('kvm', 3, 0)],
    10: [('epi', 0), ('kvm', 3, 1)],
    11: [('kvm', 3, 2)],
    12: [('kvm', 3, 3), ('kve', 3)],
    13: [('vfin', 3)],
    16: [('epi', 1)],
    17: [('kvm', 4, 0)], 18: [('kvm', 4, 1)], 19: [('kvm', 4, 2)],
    20: [('kvm', 4, 3), ('kve', 4)], 21: [('vfin', 4)],
    22: [('kvm', 5, 0)], 23: [('kvm', 5, 1)], 24: [('kvm', 5, 2)],
    25: [('kvm', 5, 3), ('kve', 5)], 26: [('vfin', 5)],
    30: [('epi', 2)],
}


def _install_profile_hook():
    """Best-effort NTFF profiling hook (the image's antenv lacks axon_hooks)."""
    try:
        import antenv
        if "antenv.axon_hooks" in sys.modules:
            return
        hooks_mod = types.ModuleType("antenv.axon_hooks")
        _h = [None]
        hooks_mod.set_axon_ntff_profile_hook = lambda h: _h.__setitem__(0, h)
        hooks_mod.get_axon_ntff_profile_hook = lambda: _h[0]
        sys.modules["antenv.axon_hooks"] = hooks_mod
        antenv.axon_hooks = hooks_mod
        from trn_agent_boot.trn_boot import _ntff_profile_via_ctypes
        hooks_mod.set_axon_ntff_profile_hook(
            _ntff_profile_via_ctypes("/opt/axon/libaxon_pjrt.so")
        )
        import concourse.bass_utils as bass_utils
        bass_utils.upload_artifacts = lambda tmpdir: f"local:{tmpdir}"
    except Exception:
        pass


def build_graph():
    import concourse.bacc as bacc
    import concourse.mybir as mybir
    import concourse.tile as tile
    from concourse import masks

    F32 = mybir.dt.float32
    BF16 = mybir.dt.bfloat16

    nc = bacc.Bacc("TRN2", target_bir_lowering=False, debug=False,
                   num_devices=NCORE)

    xT = nc.dram_tensor("xT", [NCH, 128, 8, TCH], BF16,
                        kind="ExternalInput").ap()
    wkv = nc.dram_tensor("wkv", [128, 8, 2 * H], BF16,
                         kind="ExternalInput").ap()
    wq = nc.dram_tensor("wq", [128, 8, H], BF16, kind="ExternalInput").ap()
    # numerator (64) + denominator (col 64) + pad, normalized on host
    out = nc.dram_tensor("out", [T // 2, H + 2], BF16,
                         kind="ExternalOutput").ap()
    out_r = out.rearrange("(l p) h -> p l h", p=128)

    with tile.TileContext(nc) as tc:
        import contextlib
        with contextlib.ExitStack() as ctx:
            _body(ctx, tc, nc, mybir, masks, xT, wkv, wq, out_r)

    nc.compile()
    return nc


def _body(ctx, tc, nc, mybir, masks, xT, wkv, wq, out_r):
    F32 = mybir.dt.float32
    BF16 = mybir.dt.bfloat16
    Exp = mybir.ActivationFunctionType.Exp

    const = ctx.enter_context(tc.tile_pool(name="const", bufs=1))
    big = ctx.enter_context(tc.tile_pool(name="big", bufs=1))
    vs_pool = ctx.enter_context(tc.tile_pool(name="vs", bufs=2))
    pt_pool = ctx.enter_context(tc.tile_pool(name="pt", bufs=6))
    wk_pool = ctx.enter_context(tc.tile_pool(name="wk", bufs=3))
    ps_pool = ctx.enter_context(tc.tile_pool(name="ps", bufs=2, space="PSUM"))
    po_pool = ctx.enter_context(tc.tile_pool(name="po", bufs=2, space="PSUM"))
    aux_ps = ctx.enter_context(tc.tile_pool(name="auxps", bufs=2, space="PSUM"))

    _psn = [0]
    def ps_tile(shape, dtype=None):
        _psn[0] += 1
        return ps_pool.tile(shape, dtype or F32, tag="ps", name=f"ps{_psn[0]}")

    def po_tile(shape, dtype=None):
        _psn[0] += 1
        return po_pool.tile(shape, dtype or F32, tag="po", name=f"po{_psn[0]}")

    def aux_tile(shape, dtype=None):
        _psn[0] += 1
        return aux_ps.tile(shape, dtype or F32, tag="aux", name=f"aux{_psn[0]}")

    _wkn = [0]
    def wk_tile(shape, dtype=None, tag="wk"):
        _wkn[0] += 1
        return wk_pool.tile(shape, dtype or F32, tag=tag,
                            name=f"{tag}{_wkn[0]}")

    # ---- identity first (gates the PE warmup), then weight DMAs, then
    # the first x chunk, then remaining constants and x chunks.  All
    # issue from the GpSimd queue.
    ident = const.tile([128, 128], BF16)
    masks.make_identity(nc, ident[:])

    kT_a = big.tile([128, NCH, TCH], BF16)
    vA_a = big.tile([128, NCH, 4, H + 1], BF16)
    qT_a = big.tile([128, 4, TCH], BF16)
    xcs = [big.tile([128, 8, TCH], BF16, name=f"xc{s}") for s in range(NCH)]

    # Each dma_start binds to one ~23GB/s DMA engine, so aggregate
    # bandwidth ramps with the number of transfers in flight: fan the
    # issues across four engine queues in parallel and split the
    # early-needed data fine.  Priority order: weights, chunk 0, 1, ...
    # transfers with 4KB-per-partition lines sustain ~410GB/s aggregate
    # (finer splits drop to ~250); priority order = weights, slot 0, 1, ...
    w_kv = const.tile([128, 8, 2 * H], BF16)
    w_q = const.tile([128, 8, H], BF16)
    pieces = [(w_kv[:], wkv[:]), (w_q[:], wq[:])]
    for s in range(2):                       # first two slots: quarters
        for q4 in range(4):
            pieces.append((xcs[s][:, 2 * q4:2 * q4 + 2, :],
                           xT[s][:, 2 * q4:2 * q4 + 2, :]))
    for s in range(2, 6):
        pieces.append((xcs[s][:, 0:4, :], xT[s][:, 0:4, :]))
        pieces.append((xcs[s][:, 4:8, :], xT[s][:, 4:8, :]))
    queues = [nc.gpsimd, nc.scalar, nc.sync]
    for idx, (dst, src) in enumerate(pieces):
        queues[idx % 3].dma_start(dst, src)

    ones_col = const.tile([128, 4], BF16)
    nc.gpsimd.memset(ones_col[:], 1.0)
    # triangle mask for the diagonal 128x128 sub-blocks
    tri = const.tile([128, JB], BF16)
    nc.gpsimd.memset(tri[:], 1.0)
    nc.gpsimd.affine_select(
        out=tri[:], in_=tri[:], compare_op=mybir.AluOpType.is_ge,
        fill=0.0, base=0, channel_multiplier=-1, pattern=[[1, JB]])

    # PE HAM warmup across the DMA window
    warm = ps_tile([128, 2 * TCH])
    for _ in range(64):
        nc.tensor.matmul(warm[:, 0:128], ident[:], ident[:],
                         start=True, stop=True)

    def branch(load_order, q_chunks, q_slots, n_load, unit_order, fillers):
        slot_of = {c: s for s, c in enumerate(load_order)}
        groups = sorted(q_chunks)

        pkv_of, pq_of, vt_of, po_of = {}, {}, {}, {}
        pending = [None]           # (group, pv_closure)

        def kvm(c, j):
            if j == 0:
                pkv_of[c] = aux_tile([128, TCH])
            p, s = pkv_of[c], slot_of[c]
            for dc in (2 * j, 2 * j + 1):
                nc.tensor.matmul(p[:], w_kv[:, dc, :], xcs[s][:, dc, :],
                                 start=(dc == 0), stop=(dc == 7))

        def kve(c):
            p, s = pkv_of[c], slot_of[c]
            nc.vector.tensor_copy(kT_a[0:64, s, :], p[0:64, :])
            nc.vector.tensor_copy(kT_a[64:128, s, :], kT_a[0:64, s, :])
            vt_of[c] = vs_pool.tile([64, TCH], BF16, name=f"vt{s}", tag="vt")
            nc.vector.tensor_copy(vt_of[c][:], p[64:128, :])

        def vfin(c):
            s = slot_of[c]
            ptrv = aux_tile([128, 4, H], BF16)
            for jj in range(4):
                nc.tensor.transpose(ptrv[:, jj, :],
                                    vt_of[c][:, jj * 128:(jj + 1) * 128],
                                    ident[0:64, 0:64])
            nc.vector.tensor_copy(vA_a[:, s, :, 0:H], ptrv[:])
            nc.vector.tensor_copy(vA_a[:, s, :, H], ones_col[:, 0:4])

        def qp(a, b, p):
            if p == 0:
                pq_of[(a, b)] = aux_tile([128, TCH])
            q = pq_of[(a, b)]
            for dc in range(2 * p, 2 * p + 2):
                nc.tensor.matmul(q[0:64, :], w_q[:, dc, :],
                                 xcs[q_slots[a]][:, dc, :],
                                 start=(dc == 0), stop=(dc == 7),
                                 tile_position=(0, 0), skip_group_check=True)
                nc.tensor.matmul(q[64:128, :], w_q[:, dc, :],
                                 xcs[q_slots[b]][:, dc, :],
                                 start=(dc == 0), stop=(dc == 7),
                                 tile_position=(0, 64), skip_group_check=True)

        def qe(a, b):
            q = pq_of[(a, b)]
            for g, lo in ((a, 0), (b, 64)):
                nc.vector.tensor_copy(qT_a[0:64, g, :], q[lo:lo + 64, :])
                nc.vector.tensor_copy(qT_a[64:128, g, :], qT_a[0:64, g, :])

        def flush_pv():
            if pending[0] is None:
                return
            _, pv = pending[0]
            pending[0] = None
            pv()

        def do_half(g, cu, hh):
            sj = slot_of[cu]
            diag = (cu == groups[g])
            if cu == 0 and hh == 0:
                po_of[g] = po_tile([H + 1, TCH])
            po = po_of[g]
            pp = ps_tile([128, 2 * TCH])
            for k_ in range(2):
                t_ = 2 * hh + k_
                lo = k_ * 64
                nc.tensor.matmul(
                    pp[:, k_ * TCH:(k_ + 1) * TCH],
                    kT_a[lo:lo + 64, sj, t_ * 128:(t_ + 1) * 128],
                    qT_a[lo:lo + 64, g, :],
                    start=True, stop=True, tile_position=(lo, 0))
            pt = pt_pool.tile([128, 2 * TCH], BF16)
            if diag and hh == 1:
                # only the causally-live column ranges of blocks t2,t3
                nc.scalar.activation(pt[:, 256:512], pp[:, 256:512],
                                     Exp, scale=SCALE)
                nc.scalar.activation(pt[:, 896:1024], pp[:, 896:1024],
                                     Exp, scale=SCALE)
            else:
                nc.scalar.activation(pt[:], pp[:], Exp, scale=SCALE)
            if diag:
                for k_ in range(2):
                    t_ = 2 * hh + k_
                    c0 = t_ * JB
                    nc.vector.tensor_mul(
                        pt[:, k_ * TCH + c0:k_ * TCH + c0 + JB],
                        pt[:, k_ * TCH + c0:k_ * TCH + c0 + JB], tri[:])

            def pv():
                for k_ in range(2):
                    t_ = 2 * hh + k_
                    c0 = t_ * JB if diag else 0
                    nc.tensor.matmul(po[:, c0:TCH], vA_a[:, sj, t_, :],
                                     pt[:, k_ * TCH + c0:(k_ + 1) * TCH],
                                     start=(cu == 0 and t_ == 0),
                                     stop=(cu == groups[g] and t_ == 3),
                                     skip_group_check=True)
            return (g, pv)

        def epilogue(g):
            # ship numerator + denominator; the host does the divide
            po = po_of[g]
            ot = wk_tile([H + 1, TCH], BF16, tag="ot")
            nc.vector.tensor_copy(ot[:], po[:])
            ptr2 = aux_tile([128, 4, H + 2], BF16)
            for jj in range(4):
                nc.tensor.transpose(ptr2[:, jj, 0:H + 1],
                                    ot[:, jj * 128:(jj + 1) * 128],
                                    ident[0:H + 1, 0:H + 1])
            og = wk_tile([128, 4, H + 2], BF16, tag="og")
            nc.vector.tensor_copy(og[:], ptr2[:])
            nc.sync.dma_start(out_r[:, g * 4:(g + 1) * 4, :], og[:])

        def do_op(op):
            if op[0] == 'kv0':
                kvm(0, 0); kvm(0, 1); kvm(0, 2); kvm(0, 3)
            elif op[0] == 'kvm':
                kvm(op[1], op[2])
            elif op[0] == 'kve':
                kve(op[1])
            elif op[0] == 'vfin':
                vfin(op[1])
            elif op[0] == 'qp':
                qp(op[1], op[2], op[3])
            elif op[0] == 'qe':
                qe(op[1], op[2])
            elif op[0] == 'w2':
                if n_load > 6:
                    # real data dependency (reads kT chunk 0) so the
                    # scheduler cannot hoist these issues into the
                    # critical early-DMA window and dilute slots 0-3
                    nc.gpsimd.tensor_copy(xcs[6][0:1, 0:1, 0:4],
                                          kT_a[0:1, slot_of[0], 0:4])
                for s in range(6, n_load):
                    nc.gpsimd.dma_start(xcs[s][:, 0:4, :], xT[s][:, 0:4, :])
                    nc.gpsimd.dma_start(xcs[s][:, 4:8, :], xT[s][:, 4:8, :])
            elif op[0] == 'epi':
                if pending[0] is not None and pending[0][0] == op[1]:
                    flush_pv()
                epilogue(op[1])

        for op in PRE_SCHED:
            do_op(op)
        hidx = 0
        for (g, cu) in unit_order:
            for hh in range(2):
                new_pv = do_half(g, cu, hh)
                for op in fillers.get(hidx, []):
                    do_op(op)
                flush_pv()
                pending[0] = new_pv
                hidx += 1
        flush_pv()
        epilogue(3)

    pid = nc.partition_id()
    with tc.If(pid < 4) as cmp:
        branch(EVEN_LOAD, EVEN_CHUNKS, EVEN_QSLOTS, EVEN_NLOAD,
               UO_EVEN, FILL_EVEN)
    with cmp.Else():
        branch(ODD_LOAD, ODD_CHUNKS, ODD_QSLOTS, ODD_NLOAD,
               UO_ODD, FILL_ODD)


_GRAPH = None


def _get_graph():
    global _GRAPH
    if _GRAPH is None:
        _install_profile_hook()
        _GRAPH = build_graph()
    return _GRAPH


def _in_maps(x, Wq, Wk, Wv):
    import ml_dtypes
    bf16 = ml_dtypes.bfloat16
    x = np.asarray(x, np.float32)
    wkv = np.concatenate([np.asarray(Wk, np.float32),
                          np.asarray(Wv, np.float32)], axis=1).astype(bf16)
    wkv = wkv.reshape(8, 128, 2 * H).transpose(1, 0, 2).copy()
    wq = np.asarray(Wq, np.float32).astype(bf16)
    wq = wq.reshape(8, 128, H).transpose(1, 0, 2).copy()
    maps = []
    for c in range(NCORE):
        b = c % B
        order = EVEN_LOAD if c < 4 else ODD_LOAD
        n_load = EVEN_NLOAD if c < 4 else ODD_NLOAD
        xb = x[b]                                    # [T, D]
        xTc = np.zeros((NCH, 128, 8, TCH), bf16)
        for s, gc in enumerate(order):
            if s >= n_load:
                continue
            ch = xb[gc * TCH:(gc + 1) * TCH].T       # [D, TCH]
            xTc[s] = ch.reshape(8, 128, TCH).transpose(1, 0, 2)
        maps.append({"xT": xTc, "wkv": wkv, "wq": wq})
    return maps


def _unshard(results):
    out = np.empty((B, T, H), np.float32)
    for c in range(NCORE):
        b = c % B
        chunks = EVEN_CHUNKS if c < 4 else ODD_CHUNKS
        o = np.asarray(results[c]["out"], np.float32)  # [2048, 66] num+den
        ob = o[:, 0:H] / o[:, H:H + 1]
        for g, gc in enumerate(sorted(chunks)):
            out[b, gc * TCH:(gc + 1) * TCH] = ob[g * TCH:(g + 1) * TCH]
    return out


def run_spmd(inputs, trace=False):
    """Run on 8 cores; returns (output, BassKernelResults)."""
    from concourse.bass_utils import run_bass_kernel_spmd
    nc = _get_graph()
    maps = _in_maps(inputs["x"], inputs["Wq"], inputs["Wk"], inputs["Wv"])
    res = run_bass_kernel_spmd(nc, maps, core_ids=list(range(NCORE)),
                               trace=trace)
    return _unshard(res.results), res


def _numpy_fallback(x, Wq, bq, Wk, bk, Wv, bv):
    x = np.asarray(x, np.float32)
    q = x @ Wq + bq
    k = x @ Wk + bk
    v = x @ Wv + bv
    att = np.einsum("bth,bsh->bts", q, k) / np.sqrt(np.float32(D))
    causal = np.tril(np.ones((T, T), dtype=bool))
    att = np.where(causal, att, -np.inf)
    att = att - att.max(axis=-1, keepdims=True)
    e = np.exp(att)
    att = e / e.sum(axis=-1, keepdims=True)
    return np.einsum("bts,bsh->bth", att, v).astype(np.float32)


def kernel(x, Wq, bq, Wk, bk, Wv, bv):
    if np.any(np.asarray(bq)) or np.any(np.asarray(bk)) \
            or np.any(np.asarray(bv)):
        return _numpy_fallback(x, Wq, bq, Wk, bk, Wv, bv)
    out, _ = run_spmd(dict(x=x, Wq=Wq, Wk=Wk, Wv=Wv))
    return out
